# revision 14
# baseline (speedup 1.0000x reference)
"""Trainium2 Bass kernel for nn_ColorHistograms (histogram -> banded sims -> windowed fc).

Sharding: data-parallel, one video (T=1024 frames) per NeuronCore, 8 cores, SPMD.

Per-core pipeline:
  1. 512-bin color histogram per frame, factored 512 = 32 x 16:
     DVE builds one-hots (is_equal vs iota consts, bf16 2x mode); PE contracts
     128-pixel chunks (oh16^T @ oh32) into PSUM, 4 frames col-tiled concurrently.
  2. PE transposes + compaction copies -> X2[32*slab + v32, 16*F + v16] bf16
     (exact integer counts); slab-mirror DMAs so each 32-row slab has all frames.
  3. Banded sims (|t-s| <= 50 needs only adjacent 128-blocks): 16 K=32 matmuls
     per block pair accumulate exact integer <h_t, h_s> into PSUM strips.
  4. Strips -> DRAM; skewed-AP DMA (stride 385) gathers windows[t, w] =
     sims[t, t+w-50]. The w=50 column is ||h_t||^2 (diag) for free.
  5. Normalize after the gather: inv = rsqrt(diag) via sqrt + reciprocal +
     one Newton step; winN = win * inv[t] * inv[t+w-50] (zero-padded edges
     reproduce the reference's zero padding).
  6. fc: PE transpose of winN -> matmul with host-pretransposed W_fc^T;
     bias + relu on DVE; DMA out.
"""

import numpy as np

B = 8
T_FULL = 1024
PX = 32 * 32
LOOKUP_WINDOW = 101
OUT_DIM = 128

_CACHE = {}


def build_nc(T=T_FULL, quant_offset=None, phases="all"):
    # HW DVE float->int32 output conversion rounds to nearest; the sim
    # truncates. Midpoint offsets make every value >=1/128 away from a
    # rounding boundary, so any nearest-mode is exact on HW.
    # quant_offset=(0,0) reproduces the truncating sim semantics.
    from contextlib import ExitStack

    import concourse.bass as bass
    import concourse.mybir as mybir
    from concourse import bacc
    from concourse.masks import make_identity
    from concourse.tile import TileContext

    dt = mybir.dt
    op = mybir.AluOpType
    act = mybir.ActivationFunctionType
    qoff32, qoff64 = (-15.5 / 32.0, -31.5 / 64.0) if quant_offset is None else quant_offset

    NBLK = T // 128
    NFILL = T // 64

    nc = bacc.Bacc()
    frames_d = nc.declare_dram_parameter("frames", [T, PX * 3], dt.int32, isOutput=False)
    wfct_d = nc.declare_dram_parameter("wfct", [LOOKUP_WINDOW, OUT_DIM], dt.float32, isOutput=False)
    bfc_d = nc.declare_dram_parameter("bfc", [1, OUT_DIM], dt.float32, isOutput=False)
    y_d = nc.declare_dram_parameter("y", [T, OUT_DIM], dt.float32, isOutput=True)

    strips_d = nc.dram_tensor("strips", [NBLK, 128, 384], dt.float32)
    invpad_d = nc.dram_tensor("invpad", [1, 64 + T + 64], dt.float32)

    with TileContext(nc) as tc, ExitStack() as ctx:
        singles = ctx.enter_context(tc.tile_pool(name="singles", bufs=1))
        raws = ctx.enter_context(tc.tile_pool(name="raws", bufs=3))
        preps = ctx.enter_context(tc.tile_pool(name="preps", bufs=3))
        ohs = ctx.enter_context(tc.tile_pool(name="ohs", bufs=3))
        hsts = ctx.enter_context(tc.tile_pool(name="hsts", bufs=2))
        winp = ctx.enter_context(tc.tile_pool(name="winp", bufs=3))
        allwin = ctx.enter_context(tc.tile_pool(name="allwin", bufs=2 * NBLK))
        smalls = ctx.enter_context(tc.tile_pool(name="smalls", bufs=8))
        outs = ctx.enter_context(tc.tile_pool(name="outs", bufs=3))

        # ---------------- constants ----------------
        iota32 = singles.tile([128, 32, 128], dt.bfloat16)
        nc.gpsimd.iota(iota32, pattern=[[1, 32], [0, 128]], base=0,
                       channel_multiplier=0, allow_small_or_imprecise_dtypes=True)
        iota16 = singles.tile([128, 16, 128], dt.bfloat16)
        nc.gpsimd.iota(iota16, pattern=[[1, 16], [0, 128]], base=0,
                       channel_multiplier=0, allow_small_or_imprecise_dtypes=True)
        ident_bf = singles.tile([128, 128], dt.bfloat16)
        make_identity(nc, ident_bf)
        ident_f = singles.tile([128, 128], dt.float32)
        make_identity(nc, ident_f)
        wfct_sb = singles.tile([LOOKUP_WINDOW, OUT_DIM], dt.float32)
        nc.sync.dma_start(out=wfct_sb, in_=wfct_d[:, :])
        b_rep = singles.tile([128, OUT_DIM], dt.float32)
        nc.sync.dma_start(out=b_rep, in_=bfc_d[:, :].broadcast_to([128, OUT_DIM]))
        zeros128 = singles.tile([128, 128], dt.float32)
        nc.vector.memset(zeros128, 0.0)

        # zero the never-written edges of strips + invpad
        nc.sync.dma_start(out=strips_d[0, :, 0:128], in_=zeros128)
        nc.sync.dma_start(out=strips_d[NBLK - 1, :, 256:384], in_=zeros128)
        nc.sync.dma_start(out=invpad_d[0:1, 0:64], in_=zeros128[0:1, 0:64])
        nc.sync.dma_start(out=invpad_d[0:1, 64 + T:64 + T + 64], in_=zeros128[0:1, 0:64])

        # X2[32*slab + v32, 16*F + v16]  (bin = 16*v32 + v16)
        x2 = singles.tile([128, 16 * T], dt.bfloat16)

        frames_flat = frames_d[:, :].flatten()

        # ---------------- Phase H: histograms ----------------
        hctx = ExitStack()
        ps_h = hctx.enter_context(tc.tile_pool(name="ps_h", bufs=2, space="PSUM"))
        ps_t = hctx.enter_context(tc.tile_pool(name="ps_t", bufs=2, space="PSUM"))
        for fill in range(NFILL):
            psum_h = ps_h.tile([128, 512], dt.float32)
            for ft in range(4):  # 16 frames per DVE tile
                fbase = 64 * fill + 16 * ft
                raw = raws.tile([128, 16, 24], dt.int32, tag="raw")
                # flat = f*3072 + (8p + c)*3 + ch  ->  dest [p, f, (c, ch)]
                src = bass.AP(
                    tensor=frames_flat.tensor,
                    offset=frames_flat.offset + fbase * 3072,
                    ap=[[24, 128], [3072, 16], [1, 24]],
                )
                nc.gpsimd.dma_start(out=raw, in_=src)

                # q = trunc(raw * (1/32) + off) ; int32 output cast truncates (or
                # rounds -- quant_offset compensates, selected by sim/HW check)
                q_all = preps.tile([128, 16, 24], dt.int32, tag="qall")
                nc.vector.tensor_scalar(q_all, raw, 1.0 / 32.0, qoff32,
                                        op0=op.mult, op1=op.add)
                qg6 = preps.tile([128, 16, 8], dt.int32, tag="qg6")
                rawg = raw.rearrange("p f (c ch) -> p f c ch", ch=3)[:, :, :, 1]
                nc.vector.tensor_scalar(qg6, rawg, 1.0 / 64.0, qoff64,
                                        op0=op.mult, op1=op.add)
                qv = q_all.rearrange("p f (c ch) -> p f c ch", ch=3)
                v32b = preps.tile([128, 16, 8], dt.bfloat16, tag="v32b")
                nc.vector.scalar_tensor_tensor(v32b, qv[:, :, :, 0], 4.0, qg6,
                                               op0=op.mult, op1=op.add)
                t1 = preps.tile([128, 16, 8], dt.int32, tag="t1")
                nc.vector.scalar_tensor_tensor(t1, qv[:, :, :, 1], 8.0, qv[:, :, :, 2],
                                               op0=op.mult, op1=op.add)
                v16b = preps.tile([128, 16, 8], dt.bfloat16, tag="v16b")
                nc.vector.scalar_tensor_tensor(v16b, qg6, -16.0, t1,
                                               op0=op.mult, op1=op.add)

                # one-hots [128, v, 128] ; px-col cc = 8*fl + c
                oh32 = ohs.tile([128, 32, 128], dt.bfloat16, tag="oh32")
                v32r = v32b.rearrange("p f c -> p (f c)").unsqueeze(1).broadcast_to([128, 32, 128])
                nc.vector.tensor_tensor(oh32, v32r, iota32, op.is_equal)
                oh16 = ohs.tile([128, 16, 128], dt.bfloat16, tag="oh16")
                v16r = v16b.rearrange("p f c -> p (f c)").unsqueeze(1).broadcast_to([128, 16, 128])
                nc.vector.tensor_tensor(oh16, v16r, iota16, op.is_equal)

                # hist matmuls: frame f_local = 4k + g, k = 4*ft + kq
                for kq in range(4):
                    k = 4 * ft + kq
                    for g in range(4):
                        fl = (4 * k + g) - 16 * ft
                        for c in range(8):
                            cc = 8 * fl + c
                            nc.tensor.matmul(
                                psum_h[32 * g:32 * g + 16, 32 * k:32 * k + 32],
                                lhsT=oh16[:, :, cc],
                                rhs=oh32[:, :, cc],
                                start=(c == 0), stop=(c == 7),
                                tile_position=(0, 32 * g),
                            )
            hst = hsts.tile([128, 512], dt.bfloat16, tag="hst")
            # rows 16..31 of each 32-block were never matmul-written; zero the
            # staging tile and copy only the valid row bands
            nc.gpsimd.memset(hst, 0.0)
            for g in range(4):
                nc.scalar.activation(hst[32 * g:32 * g + 16, :],
                                     psum_h[32 * g:32 * g + 16, :], act.Copy)
            for tau in range(4):
                pst = ps_t.tile([128, 128], dt.bfloat16, tag="pst")
                nc.tensor.transpose(pst, hst[:, 128 * tau:128 * tau + 128], ident_bf)
                for kap in range(4):
                    src3 = pst[32 * kap:32 * kap + 32, :].rearrange(
                        "p (g w) -> p g w", g=4)[:, :, 0:16]
                    off = 16 * (64 * fill + 16 * tau + 4 * kap)
                    dst3 = x2[32 * kap:32 * kap + 32, off:off + 64].rearrange(
                        "p (g v) -> p g v", g=4)
                    nc.scalar.activation(dst3, src3, act.Copy)

        hctx.close()

        # ---------------- mirror slabs ----------------
        for kap in range(4):
            srcv = x2[32 * kap:32 * kap + 32, :].rearrange(
                "p (m q) -> p m q", q=256)[:, :, 64 * kap:64 * kap + 64]
            for rho in range(4):
                if rho == kap:
                    continue
                dstv = x2[32 * rho:32 * rho + 32, :].rearrange(
                    "p (m q) -> p m q", q=256)[:, :, 64 * kap:64 * kap + 64]
                nc.sync.dma_start(out=dstv, in_=srcv)

        # ---------------- Phase S: banded sims ----------------
        sctx = ExitStack()
        ps_s = sctx.enter_context(tc.tile_pool(name="ps_s", bufs=4, space="PSUM"))
        x2v = x2.rearrange("p (f v) -> p f v", v=16)
        for i in range(NBLK):
            js = [j for j in (i - 1, i, i + 1) if 0 <= j < NBLK]
            if phases == "diag":
                js = [i]
            strip_sb = hsts.tile([128, 384], dt.float32, tag="strip_sb")
            for j in js:
                jj = j - i + 1
                rho = (i + jj) % 4
                # one PSUM bank per pair: concurrent row-group matmul streams
                # to a shared bank fault the PSUM
                psum_s = ps_s.tile([128, 128], dt.float32, tag="psum_s")
                for m in range(16):
                    nc.tensor.matmul(
                        psum_s,
                        lhsT=x2v[32 * rho:32 * rho + 32, 128 * i:128 * i + 128, m],
                        rhs=x2v[32 * rho:32 * rho + 32, 128 * j:128 * j + 128, m],
                        start=(m == 0), stop=(m == 15),
                        tile_position=(32 * rho, 0),
                    )
                nc.scalar.activation(
                    strip_sb[:, 128 * jj:128 * jj + 128], psum_s, act.Copy,
                )
            jj0, jj1 = js[0] - i + 1, js[-1] - i + 1
            nc.sync.dma_start(
                out=strips_d[i, :, 128 * jj0:128 * (jj1 + 1)],
                in_=strip_sb[:, 128 * jj0:128 * (jj1 + 1)],
            )

        sctx.close()

        # ---------------- windows gather + inv ----------------
        strips_flat = strips_d[:, :, :].flatten()
        invpad_flat = invpad_d[:, :].flatten()
        wins = []
        for i in range(NBLK):
            win = allwin.tile([128, LOOKUP_WINDOW], dt.float32, tag=f"win{i}")
            src = bass.AP(
                tensor=strips_flat.tensor,
                offset=strips_flat.offset + i * 128 * 384 + 78,
                ap=[[385, 128], [1, LOOKUP_WINDOW]],
            )
            nc.sync.dma_start(out=win, in_=src)
            # inv = rsqrt(win[:, 50]) + one Newton step
            s0 = smalls.tile([128, 1], dt.float32, tag="s0")
            nc.scalar.activation(s0, win[:, 50:51], act.Sqrt)
            b0 = smalls.tile([128, 1], dt.float32, tag="b0")
            nc.vector.reciprocal(b0, s0)
            c0 = smalls.tile([128, 1], dt.float32, tag="c0")
            nc.vector.tensor_tensor(c0, b0, b0, op.mult)
            d0 = smalls.tile([128, 1], dt.float32, tag="d0")
            nc.vector.tensor_tensor(d0, c0, win[:, 50:51], op.mult)
            e0 = smalls.tile([128, 1], dt.float32, tag="e0")
            nc.vector.tensor_scalar(e0, d0, -0.5, 1.5, op0=op.mult, op1=op.add)
            inv = allwin.tile([128, 1], dt.float32, tag=f"inv{i}")
            nc.vector.tensor_tensor(inv, b0, e0, op.mult)
            dst = bass.AP(
                tensor=invpad_flat.tensor,
                offset=invpad_flat.offset + 64 + 128 * i,
                ap=[[1, 128], [1, 1]],
            )
            nc.sync.dma_start(out=dst, in_=inv)
            wins.append((win, inv))

        # ---------------- Phase F: normalize + fc ----------------
        if phases == "nofc":
            for i in range(NBLK):
                nc.sync.dma_start(out=y_d[128 * i:128 * i + 128, :], in_=zeros128)
            return nc
        fctx = ExitStack()
        ps_f = fctx.enter_context(tc.tile_pool(name="ps_f", bufs=2, space="PSUM"))
        for i in range(NBLK):
            win, inv = wins[i]
            invwin = winp.tile([128, LOOKUP_WINDOW], dt.float32, tag="invwin")
            src = bass.AP(
                tensor=invpad_flat.tensor,
                offset=invpad_flat.offset + 64 + 128 * i - 50,
                ap=[[1, 128], [1, LOOKUP_WINDOW]],
            )
            nc.sync.dma_start(out=invwin, in_=src)
            wn1 = winp.tile([128, LOOKUP_WINDOW], dt.float32, tag="wn1")
            nc.vector.tensor_scalar(wn1, win, inv, None, op0=op.mult)
            winN = winp.tile([128, LOOKUP_WINDOW], dt.float32, tag="winN")
            nc.vector.tensor_tensor(winN, wn1, invwin, op.mult)
            ps_w = ps_f.tile([LOOKUP_WINDOW, 128], dt.float32, tag="psw")
            nc.tensor.transpose(ps_w, winN, ident_f)
            wT = winp.tile([LOOKUP_WINDOW, 128], dt.float32, tag="wT")
            nc.scalar.activation(wT, ps_w, act.Copy)
            ps_o = ps_f.tile([128, OUT_DIM], dt.float32, tag="pso")
            nc.tensor.matmul(ps_o, lhsT=wT, rhs=wfct_sb, start=True, stop=True)
            s2 = outs.tile([128, OUT_DIM], dt.float32, tag="s2")
            nc.vector.tensor_tensor(s2, ps_o, b_rep, op.add)
            yt = outs.tile([128, OUT_DIM], dt.float32, tag="yt")
            nc.vector.tensor_scalar_max(yt, s2, 0.0)
            nc.sync.dma_start(out=y_d[128 * i:128 * i + 128, :], in_=yt)
        fctx.close()

    return nc


def _host_inputs(frames, W_fc, b_fc):
    wfct = np.ascontiguousarray(np.asarray(W_fc, np.float32).T)
    bfc = np.ascontiguousarray(np.asarray(b_fc, np.float32)[None, :])
    f = np.asarray(frames)
    return [
        {
            "frames": np.ascontiguousarray(f[i].reshape(f.shape[1], PX * 3), dtype=np.int32),
            "wfct": wfct,
            "bfc": bfc,
        }
        for i in range(f.shape[0])
    ]


def kernel(frames, W_fc, b_fc):
    from concourse.bass_utils import run_bass_kernel_spmd

    if "nc" not in _CACHE:
        nc = build_nc()
        nc.finalize()
        _CACHE["nc"] = nc
    nc = _CACHE["nc"]
    in_maps = _host_inputs(frames, W_fc, b_fc)
    res = run_bass_kernel_spmd(nc, in_maps, list(range(B)))
    out = np.stack([res.results[i]["y"] for i in range(B)], axis=0)
    return out.astype(np.float32)


# revision 15
# speedup vs baseline: 1.1483x; 1.1483x over previous
"""Trainium2 Bass kernel for nn_ColorHistograms (histogram -> banded sims -> windowed fc).

Sharding: data-parallel, one video (T=1024 frames) per NeuronCore, 8 cores, SPMD.

Per-core pipeline:
  1. 512-bin color histogram per frame, factored 512 = 32 x 16:
     DVE builds one-hots (is_equal vs iota consts, bf16 2x mode); PE contracts
     128-pixel chunks (oh16^T @ oh32) into PSUM, 4 frames col-tiled concurrently.
  2. PE transposes + compaction copies -> X2[32*slab + v32, 16*F + v16] bf16
     (exact integer counts); slab-mirror DMAs so each 32-row slab has all frames.
  3. Banded sims (|t-s| <= 50 needs only adjacent 128-blocks): 16 K=32 matmuls
     per block pair accumulate exact integer <h_t, h_s> into PSUM strips.
  4. Strips -> DRAM; skewed-AP DMA (stride 385) gathers windows[t, w] =
     sims[t, t+w-50]. The w=50 column is ||h_t||^2 (diag) for free.
  5. Normalize after the gather: inv = rsqrt(diag) via sqrt + reciprocal +
     one Newton step; winN = win * inv[t] * inv[t+w-50] (zero-padded edges
     reproduce the reference's zero padding).
  6. fc: PE transpose of winN -> matmul with host-pretransposed W_fc^T;
     bias + relu on DVE; DMA out.
"""

import numpy as np

B = 8
T_FULL = 1024
PX = 32 * 32
LOOKUP_WINDOW = 101
OUT_DIM = 128

_CACHE = {}


def build_nc(T=T_FULL, quant_offset=None, phases="all"):
    # HW DVE float->int32 output conversion rounds to nearest; the sim
    # truncates. Midpoint offsets make every value >=1/128 away from a
    # rounding boundary, so any nearest-mode is exact on HW.
    # quant_offset=(0,0) reproduces the truncating sim semantics.
    from contextlib import ExitStack

    import concourse.bass as bass
    import concourse.mybir as mybir
    from concourse import bacc
    from concourse.masks import make_identity
    from concourse.tile import TileContext

    dt = mybir.dt
    op = mybir.AluOpType
    act = mybir.ActivationFunctionType
    qoff32, qoff64 = (-15.5 / 32.0, -31.5 / 64.0) if quant_offset is None else quant_offset

    NBLK = T // 128
    NFILL = T // 64

    nc = bacc.Bacc()
    frames_d = nc.declare_dram_parameter("frames", [T, PX * 3], dt.int32, isOutput=False)
    wfct_d = nc.declare_dram_parameter("wfct", [LOOKUP_WINDOW, OUT_DIM], dt.float32, isOutput=False)
    bfc_d = nc.declare_dram_parameter("bfc", [1, OUT_DIM], dt.float32, isOutput=False)
    y_d = nc.declare_dram_parameter("y", [T, OUT_DIM], dt.float32, isOutput=True)

    strips_d = nc.dram_tensor("strips", [NBLK, 128, 384], dt.float32)
    invpad_d = nc.dram_tensor("invpad", [1, 64 + T + 64], dt.float32)

    with TileContext(nc) as tc, ExitStack() as ctx:
        singles = ctx.enter_context(tc.tile_pool(name="singles", bufs=1))
        raws = ctx.enter_context(tc.tile_pool(name="raws", bufs=3))
        preps = ctx.enter_context(tc.tile_pool(name="preps", bufs=3))
        ohs = ctx.enter_context(tc.tile_pool(name="ohs", bufs=3))
        hsts = ctx.enter_context(tc.tile_pool(name="hsts", bufs=2))
        winp = ctx.enter_context(tc.tile_pool(name="winp", bufs=3))
        allwin = ctx.enter_context(tc.tile_pool(name="allwin", bufs=2 * NBLK))
        smalls = ctx.enter_context(tc.tile_pool(name="smalls", bufs=8))
        outs = ctx.enter_context(tc.tile_pool(name="outs", bufs=3))

        # ---------------- constants ----------------
        iota32 = singles.tile([128, 32, 128], dt.bfloat16)
        nc.gpsimd.iota(iota32, pattern=[[1, 32], [0, 128]], base=0,
                       channel_multiplier=0, allow_small_or_imprecise_dtypes=True)
        iota16 = singles.tile([128, 16, 128], dt.bfloat16)
        nc.gpsimd.iota(iota16, pattern=[[1, 16], [0, 128]], base=0,
                       channel_multiplier=0, allow_small_or_imprecise_dtypes=True)
        ident_bf = singles.tile([128, 128], dt.bfloat16)
        make_identity(nc, ident_bf)
        ident_f = singles.tile([128, 128], dt.float32)
        make_identity(nc, ident_f)
        wfct_sb = singles.tile([LOOKUP_WINDOW, OUT_DIM], dt.float32)
        nc.sync.dma_start(out=wfct_sb, in_=wfct_d[:, :])
        b_rep = singles.tile([128, OUT_DIM], dt.float32)
        nc.sync.dma_start(out=b_rep, in_=bfc_d[:, :].broadcast_to([128, OUT_DIM]))
        zeros128 = singles.tile([128, 128], dt.float32)
        nc.vector.memset(zeros128, 0.0)

        # zero the never-written edges of strips + invpad
        nc.sync.dma_start(out=strips_d[0, :, 0:128], in_=zeros128)
        nc.sync.dma_start(out=strips_d[NBLK - 1, :, 256:384], in_=zeros128)
        nc.sync.dma_start(out=invpad_d[0:1, 0:64], in_=zeros128[0:1, 0:64])
        nc.sync.dma_start(out=invpad_d[0:1, 64 + T:64 + T + 64], in_=zeros128[0:1, 0:64])

        # X2[32*slab + v32, 16*F + v16]  (bin = 16*v32 + v16)
        x2 = singles.tile([128, 16 * T], dt.bfloat16)

        frames_flat = frames_d[:, :].flatten()

        # ---------------- Phase H: histograms ----------------
        hctx = ExitStack()
        ps_h = hctx.enter_context(tc.tile_pool(name="ps_h", bufs=2, space="PSUM"))
        ps_t = hctx.enter_context(tc.tile_pool(name="ps_t", bufs=2, space="PSUM"))
        for fill in range(NFILL):
            psum_h = ps_h.tile([128, 512], dt.float32)
            for ft in range(4):  # 16 frames per DVE tile
                fbase = 64 * fill + 16 * ft
                raw = raws.tile([128, 16, 24], dt.int32, tag="raw")
                # flat = f*3072 + (8p + c)*3 + ch  ->  dest [p, f, (c, ch)]
                src = bass.AP(
                    tensor=frames_flat.tensor,
                    offset=frames_flat.offset + fbase * 3072,
                    ap=[[24, 128], [3072, 16], [1, 24]],
                )
                nc.sync.dma_start(out=raw, in_=src)

                # q = trunc(raw * (1/32) + off) ; int32 output cast truncates (or
                # rounds -- quant_offset compensates, selected by sim/HW check)
                q_all = preps.tile([128, 16, 24], dt.int32, tag="qall")
                nc.vector.tensor_scalar(q_all, raw, 1.0 / 32.0, qoff32,
                                        op0=op.mult, op1=op.add)
                qg6 = preps.tile([128, 16, 8], dt.int32, tag="qg6")
                rawg = raw.rearrange("p f (c ch) -> p f c ch", ch=3)[:, :, :, 1]
                nc.vector.tensor_scalar(qg6, rawg, 1.0 / 64.0, qoff64,
                                        op0=op.mult, op1=op.add)
                qv = q_all.rearrange("p f (c ch) -> p f c ch", ch=3)
                v32b = preps.tile([128, 16, 8], dt.bfloat16, tag="v32b")
                nc.vector.scalar_tensor_tensor(v32b, qv[:, :, :, 0], 4.0, qg6,
                                               op0=op.mult, op1=op.add)
                t1 = preps.tile([128, 16, 8], dt.int32, tag="t1")
                nc.vector.scalar_tensor_tensor(t1, qv[:, :, :, 1], 8.0, qv[:, :, :, 2],
                                               op0=op.mult, op1=op.add)
                v16b = preps.tile([128, 16, 8], dt.bfloat16, tag="v16b")
                nc.vector.scalar_tensor_tensor(v16b, qg6, -16.0, t1,
                                               op0=op.mult, op1=op.add)

                # one-hots [128, v, 128] ; px-col cc = 8*fl + c
                oh32 = ohs.tile([128, 32, 128], dt.bfloat16, tag="oh32")
                v32r = v32b.rearrange("p f c -> p (f c)").unsqueeze(1).broadcast_to([128, 32, 128])
                nc.vector.tensor_tensor(oh32, v32r, iota32, op.is_equal)
                oh16 = ohs.tile([128, 16, 128], dt.bfloat16, tag="oh16")
                v16r = v16b.rearrange("p f c -> p (f c)").unsqueeze(1).broadcast_to([128, 16, 128])
                nc.vector.tensor_tensor(oh16, v16r, iota16, op.is_equal)

                # hist matmuls: frame f_local = 4k + g, k = 4*ft + kq
                for kq in range(4):
                    k = 4 * ft + kq
                    for g in range(4):
                        fl = (4 * k + g) - 16 * ft
                        for c in range(8):
                            cc = 8 * fl + c
                            nc.tensor.matmul(
                                psum_h[32 * g:32 * g + 16, 32 * k:32 * k + 32],
                                lhsT=oh16[:, :, cc],
                                rhs=oh32[:, :, cc],
                                start=(c == 0), stop=(c == 7),
                                tile_position=(0, 32 * g),
                            )
            hst = hsts.tile([128, 512], dt.bfloat16, tag="hst")
            # rows 16..31 of each 32-block were never matmul-written; zero the
            # staging tile and copy only the valid row bands
            nc.gpsimd.memset(hst, 0.0)
            for g in range(4):
                nc.scalar.activation(hst[32 * g:32 * g + 16, :],
                                     psum_h[32 * g:32 * g + 16, :], act.Copy)
            for tau in range(4):
                pst = ps_t.tile([128, 128], dt.bfloat16, tag="pst")
                nc.tensor.transpose(pst, hst[:, 128 * tau:128 * tau + 128], ident_bf)
                for kap in range(4):
                    src3 = pst[32 * kap:32 * kap + 32, :].rearrange(
                        "p (g w) -> p g w", g=4)[:, :, 0:16]
                    off = 16 * (64 * fill + 16 * tau + 4 * kap)
                    dst3 = x2[32 * kap:32 * kap + 32, off:off + 64].rearrange(
                        "p (g v) -> p g v", g=4)
                    nc.scalar.activation(dst3, src3, act.Copy)

        hctx.close()

        # ---------------- mirror slabs ----------------
        for kap in range(4):
            srcv = x2[32 * kap:32 * kap + 32, :].rearrange(
                "p (m q) -> p m q", q=256)[:, :, 64 * kap:64 * kap + 64]
            for rho in range(4):
                if rho == kap:
                    continue
                dstv = x2[32 * rho:32 * rho + 32, :].rearrange(
                    "p (m q) -> p m q", q=256)[:, :, 64 * kap:64 * kap + 64]
                nc.sync.dma_start(out=dstv, in_=srcv)

        # ---------------- Phase S: banded sims ----------------
        sctx = ExitStack()
        ps_s = sctx.enter_context(tc.tile_pool(name="ps_s", bufs=4, space="PSUM"))
        x2v = x2.rearrange("p (f v) -> p f v", v=16)
        for i in range(NBLK):
            js = [j for j in (i - 1, i, i + 1) if 0 <= j < NBLK]
            if phases == "diag":
                js = [i]
            strip_sb = hsts.tile([128, 384], dt.float32, tag="strip_sb")
            for j in js:
                jj = j - i + 1
                rho = (i + jj) % 4
                # one PSUM bank per pair: concurrent row-group matmul streams
                # to a shared bank fault the PSUM
                psum_s = ps_s.tile([128, 128], dt.float32, tag="psum_s")
                for m in range(16):
                    nc.tensor.matmul(
                        psum_s,
                        lhsT=x2v[32 * rho:32 * rho + 32, 128 * i:128 * i + 128, m],
                        rhs=x2v[32 * rho:32 * rho + 32, 128 * j:128 * j + 128, m],
                        start=(m == 0), stop=(m == 15),
                        tile_position=(32 * rho, 0),
                    )
                nc.scalar.activation(
                    strip_sb[:, 128 * jj:128 * jj + 128], psum_s, act.Copy,
                )
            jj0, jj1 = js[0] - i + 1, js[-1] - i + 1
            nc.sync.dma_start(
                out=strips_d[i, :, 128 * jj0:128 * (jj1 + 1)],
                in_=strip_sb[:, 128 * jj0:128 * (jj1 + 1)],
            )

        sctx.close()

        # ---------------- windows gather + inv ----------------
        strips_flat = strips_d[:, :, :].flatten()
        invpad_flat = invpad_d[:, :].flatten()
        wins = []
        for i in range(NBLK):
            win = allwin.tile([128, LOOKUP_WINDOW], dt.float32, tag=f"win{i}")
            src = bass.AP(
                tensor=strips_flat.tensor,
                offset=strips_flat.offset + i * 128 * 384 + 78,
                ap=[[385, 128], [1, LOOKUP_WINDOW]],
            )
            nc.sync.dma_start(out=win, in_=src)
            # inv = rsqrt(win[:, 50]) + one Newton step
            s0 = smalls.tile([128, 1], dt.float32, tag="s0")
            nc.scalar.activation(s0, win[:, 50:51], act.Sqrt)
            b0 = smalls.tile([128, 1], dt.float32, tag="b0")
            nc.vector.reciprocal(b0, s0)
            c0 = smalls.tile([128, 1], dt.float32, tag="c0")
            nc.vector.tensor_tensor(c0, b0, b0, op.mult)
            d0 = smalls.tile([128, 1], dt.float32, tag="d0")
            nc.vector.tensor_tensor(d0, c0, win[:, 50:51], op.mult)
            e0 = smalls.tile([128, 1], dt.float32, tag="e0")
            nc.vector.tensor_scalar(e0, d0, -0.5, 1.5, op0=op.mult, op1=op.add)
            inv = allwin.tile([128, 1], dt.float32, tag=f"inv{i}")
            nc.vector.tensor_tensor(inv, b0, e0, op.mult)
            dst = bass.AP(
                tensor=invpad_flat.tensor,
                offset=invpad_flat.offset + 64 + 128 * i,
                ap=[[1, 128], [1, 1]],
            )
            nc.sync.dma_start(out=dst, in_=inv)
            wins.append((win, inv))

        # ---------------- Phase F: normalize + fc ----------------
        if phases == "nofc":
            for i in range(NBLK):
                nc.sync.dma_start(out=y_d[128 * i:128 * i + 128, :], in_=zeros128)
            return nc
        fctx = ExitStack()
        ps_f = fctx.enter_context(tc.tile_pool(name="ps_f", bufs=2, space="PSUM"))
        for i in range(NBLK):
            win, inv = wins[i]
            invwin = winp.tile([128, LOOKUP_WINDOW], dt.float32, tag="invwin")
            src = bass.AP(
                tensor=invpad_flat.tensor,
                offset=invpad_flat.offset + 64 + 128 * i - 50,
                ap=[[1, 128], [1, LOOKUP_WINDOW]],
            )
            nc.sync.dma_start(out=invwin, in_=src)
            wn1 = winp.tile([128, LOOKUP_WINDOW], dt.float32, tag="wn1")
            nc.vector.tensor_scalar(wn1, win, inv, None, op0=op.mult)
            winN = winp.tile([128, LOOKUP_WINDOW], dt.float32, tag="winN")
            nc.vector.tensor_tensor(winN, wn1, invwin, op.mult)
            ps_w = ps_f.tile([LOOKUP_WINDOW, 128], dt.float32, tag="psw")
            nc.tensor.transpose(ps_w, winN, ident_f)
            wT = winp.tile([LOOKUP_WINDOW, 128], dt.float32, tag="wT")
            nc.scalar.activation(wT, ps_w, act.Copy)
            ps_o = ps_f.tile([128, OUT_DIM], dt.float32, tag="pso")
            nc.tensor.matmul(ps_o, lhsT=wT, rhs=wfct_sb, start=True, stop=True)
            s2 = outs.tile([128, OUT_DIM], dt.float32, tag="s2")
            nc.vector.tensor_tensor(s2, ps_o, b_rep, op.add)
            yt = outs.tile([128, OUT_DIM], dt.float32, tag="yt")
            nc.vector.tensor_scalar_max(yt, s2, 0.0)
            nc.sync.dma_start(out=y_d[128 * i:128 * i + 128, :], in_=yt)
        fctx.close()

    return nc


def _host_inputs(frames, W_fc, b_fc):
    wfct = np.ascontiguousarray(np.asarray(W_fc, np.float32).T)
    bfc = np.ascontiguousarray(np.asarray(b_fc, np.float32)[None, :])
    f = np.asarray(frames)
    return [
        {
            "frames": np.ascontiguousarray(f[i].reshape(f.shape[1], PX * 3), dtype=np.int32),
            "wfct": wfct,
            "bfc": bfc,
        }
        for i in range(f.shape[0])
    ]


def kernel(frames, W_fc, b_fc):
    from concourse.bass_utils import run_bass_kernel_spmd

    if "nc" not in _CACHE:
        nc = build_nc()
        nc.finalize()
        _CACHE["nc"] = nc
    nc = _CACHE["nc"]
    in_maps = _host_inputs(frames, W_fc, b_fc)
    res = run_bass_kernel_spmd(nc, in_maps, list(range(B)))
    out = np.stack([res.results[i]["y"] for i in range(B)], axis=0)
    return out.astype(np.float32)
